# revision 1
# baseline (speedup 1.0000x reference)
"""Trainium2 Bass kernel for nn_Ensemble (dense MLP ensemble, E=8, B=65536).

v10 = v8 (layer-blocked, NB=4) with the PSUM pipeline rebuilt after phase
isolation showed the L2 phase alone costs ~170us (= the whole kernel):
ps2 was single-buffered, so every L2 matmul-pair sat in a semaphore
round-trip with its DVE drain.

  - ONE shared PSUM pool, 2 slots x [128,2048] f32 (4 banks each, 8 total).
    The layer phases time-share it; slot rotation gives every phase
    double buffering.
  - 2-tick groups: 4 matmuls (2048 psum cols) then ONE wide drain
    instruction -> half the drains, half the semaphore hops.
  - drain engines: L1 h1 split Act(lo half)/DVE(hi half); L2 h2 full on
    DVE (Act idle in L2 phase); L3 o on DVE; o out-DMA + x in-DMA on the
    SP ring as in v8.
"""

import numpy as np
import ml_dtypes

BF16 = ml_dtypes.bfloat16

E = 8
B = 65536
HB = B // 2
IN = 64
AC = 16
H = 128
OUT = 48
OUTP = 64

NT = 512
SS = 512
NB = 4
T0 = (HB // SS) // NB   # 16 ticks per block
G = 2                   # ticks per psum group

XW = 4096
OW = 4096
XBUFS = 3
OBUFS = 2

_CACHED = None


def _build_nc(reps=None):
    import contextlib
    import concourse.bacc as bacc
    import concourse.mybir as mybir
    import concourse.tile as tile

    f32 = mybir.dt.float32
    bf16 = mybir.dt.bfloat16
    AF = mybir.ActivationFunctionType
    ALU = mybir.AluOpType

    nc = bacc.Bacc("TRN2", target_bir_lowering=False)

    x_d = nc.dram_tensor("x", [128, HB], bf16, kind="ExternalInput")
    w1_d = nc.dram_tensor("w1p", [128, H], bf16, kind="ExternalInput")
    w2_d = nc.dram_tensor("w2", [H, H], bf16, kind="ExternalInput")
    w3_d = nc.dram_tensor("w3p", [H, OUTP], bf16, kind="ExternalInput")
    b1_d = nc.dram_tensor("b1v", [H, 1], f32, kind="ExternalInput")
    b2_d = nc.dram_tensor("b2v", [H, 1], f32, kind="ExternalInput")
    b3_d = nc.dram_tensor("b3v", [128, 1], f32, kind="ExternalInput")
    out_d = nc.dram_tensor("out", [128, HB], bf16, kind="ExternalOutput")

    BW = T0 * SS          # x cols per block (8192)
    HW_ = T0 * 2 * NT     # h cols per block (16384)
    GW = G * 2 * NT       # psum cols per group (2048)

    with tile.TileContext(nc) as tc:
        with (
            tc.tile_pool(name="consts", bufs=1) as consts,
            tc.tile_pool(name="xp", bufs=XBUFS) as xp,
            tc.tile_pool(name="h1p", bufs=2) as h1pool,
            tc.tile_pool(name="h2p", bufs=2) as h2pool,
            tc.tile_pool(name="osb", bufs=OBUFS) as opool,
            tc.tile_pool(name="ps", bufs=2, space="PSUM") as psp,
        ):
            w1_sb = consts.tile([128, H], bf16)
            w2_sb = consts.tile([H, H], bf16)
            w3_sb = consts.tile([H, OUTP], bf16)
            b1_sb = consts.tile([H, 1], f32)
            b2_sb = consts.tile([H, 1], f32)
            b3_sb = consts.tile([128, 1], f32)
            nc.sync.dma_start(out=w1_sb, in_=w1_d[:])
            nc.sync.dma_start(out=w2_sb, in_=w2_d[:])
            nc.sync.dma_start(out=w3_sb, in_=w3_d[:])
            nc.sync.dma_start(out=b1_sb, in_=b1_d[:])
            nc.sync.dma_start(out=b2_sb, in_=b2_d[:])
            nc.sync.dma_start(out=b3_sb, in_=b3_d[:])

            NG = T0 // G          # groups per block phase (8)

            loop = (tc.For_i(0, reps, 1, hint_engines=(mybir.EngineType.PE,))
                    if reps is not None else contextlib.nullcontext())
            with loop:
                h1s = {}
                h2s = {}
                for c in range(NB + 2):
                    # ---- in-DMAs for block c --------------------------
                    if c < NB:
                        x_ts = []
                        for k in range(BW // XW):
                            x_t = xp.tile([128, XW], bf16, name="x_t")
                            nc.sync.dma_start(
                                out=x_t,
                                in_=x_d[:, c * BW + k * XW:
                                        c * BW + (k + 1) * XW])
                            x_ts.append(x_t)

                    # ---- L1 phase: block c ----------------------------
                    if c < NB:
                        h1blk = h1pool.tile([128, HW_], bf16, name="h1blk")
                        for g in range(NG):
                            h1ps = psp.tile([128, GW], f32, name="h1ps", tag="gps")
                            for j in range(G):
                                i = g * G + j
                                x_t = x_ts[(i * SS) // XW]
                                xo = (i * SS) % XW
                                po = j * 2 * NT
                                nc.tensor.matmul(
                                    h1ps[:, po:po + NT], w1_sb[0:64, :],
                                    x_t[0:64, xo:xo + NT],
                                    start=True, stop=True)
                                nc.tensor.matmul(
                                    h1ps[:, po + NT:po + 2 * NT],
                                    w1_sb[64:128, :],
                                    x_t[64:128, xo:xo + NT],
                                    start=True, stop=True)
                            ho = g * GW
                            hw2 = GW // 2
                            nc.scalar.activation(
                                h1blk[:, ho:ho + hw2], h1ps[:, 0:hw2],
                                AF.Relu, bias=b1_sb)
                            nc.vector.tensor_scalar(
                                h1blk[:, ho + hw2:ho + GW],
                                h1ps[:, hw2:GW], b1_sb, 0.0,
                                op0=ALU.add, op1=ALU.max)
                        h1s[c] = h1blk

                    # ---- L2 phase: block c-1 --------------------------
                    if 1 <= c <= NB:
                        h1blk = h1s.pop(c - 1)
                        h2blk = h2pool.tile([128, HW_], bf16, name="h2blk")
                        for g in range(NG):
                            h2ps = psp.tile([128, GW], f32, name="h2ps", tag="gps")
                            for j in range(G):
                                ho = (g * G + j) * 2 * NT
                                po = j * 2 * NT
                                nc.tensor.matmul(
                                    h2ps[:, po:po + NT], w2_sb,
                                    h1blk[:, ho:ho + NT],
                                    start=True, stop=True)
                                nc.tensor.matmul(
                                    h2ps[:, po + NT:po + 2 * NT], w2_sb,
                                    h1blk[:, ho + NT:ho + 2 * NT],
                                    start=True, stop=True)
                            ho = g * GW
                            nc.vector.tensor_scalar(
                                h2blk[:, ho:ho + GW], h2ps, b2_sb, 0.0,
                                op0=ALU.add, op1=ALU.max)
                        h2s[c - 1] = h2blk

                    # ---- L3 phase: block c-2, out-DMAs inline ---------
                    if c >= 2:
                        b = c - 2
                        h2blk = h2s.pop(b)
                        o_t = None
                        for g in range(NG):
                            ops = psp.tile([128, GW], f32, name="ops", tag="gps")[:, 0:GW // 2]
                            for j in range(G):
                                i = g * G + j
                                ho = i * 2 * NT
                                po = j * NT
                                nc.tensor.matmul(
                                    ops[0:OUTP, po:po + NT], w3_sb,
                                    h2blk[:, ho:ho + NT],
                                    start=True, stop=True,
                                    tile_position=(0, 0))
                                nc.tensor.matmul(
                                    ops[OUTP:128, po:po + NT], w3_sb,
                                    h2blk[:, ho + NT:ho + 2 * NT],
                                    start=True, stop=True,
                                    tile_position=(0, OUTP))
                            gcol = g * G * SS       # col offset in block
                            if gcol % OW == 0:
                                o_t = opool.tile([128, OW], bf16, name="o_t")
                            oo = gcol % OW
                            nc.vector.tensor_scalar_add(
                                o_t[:, oo:oo + G * SS], ops, b3_sb)
                            if (gcol + G * SS) % OW == 0:
                                oc = b * BW + gcol + G * SS - OW
                                nc.sync.dma_start(
                                    out=out_d[:, oc:oc + OW], in_=o_t)

    nc.compile()
    return nc


def _get_nc():
    global _CACHED
    if _CACHED is None:
        _CACHED = _build_nc()
    return _CACHED


def _prep_member(x_e, W1_e, b1_e, W2_e, b2_e, W3_e, b3_e):
    xt = np.ascontiguousarray(np.asarray(x_e).T)      # [64, B] f32
    np.clip(xt[IN - AC:IN], -1.0, 1.0, out=xt[IN - AC:IN])
    X = np.empty((128, HB), dtype=BF16)
    X[0:64] = xt[:, :HB]
    X[64:128] = xt[:, HB:]

    w1p = np.empty((128, H), dtype=BF16)
    w1p[0:64] = W1_e
    w1p[64:128] = W1_e
    w2 = W2_e.astype(BF16)
    w3p = np.zeros((H, OUTP), dtype=BF16)
    w3p[:, :OUT] = W3_e
    b1v = np.ascontiguousarray(b1_e.astype(np.float32).reshape(H, 1))
    b2v = np.ascontiguousarray(b2_e.astype(np.float32).reshape(H, 1))
    b3v = np.zeros((128, 1), dtype=np.float32)
    b3v[0:OUT, 0] = b3_e
    b3v[OUTP:OUTP + OUT, 0] = b3_e
    return {"x": X, "w1p": w1p, "w2": w2, "w3p": w3p,
            "b1v": b1v, "b2v": b2v, "b3v": b3v}


def kernel(**inputs):
    from concourse.bass_utils import run_bass_kernel_spmd

    x = np.asarray(inputs["inputs"], dtype=np.float32).reshape(E, B, IN)
    W1 = np.asarray(inputs["W1"], dtype=np.float32)
    b1 = np.asarray(inputs["b1"], dtype=np.float32)
    W2 = np.asarray(inputs["W2"], dtype=np.float32)
    b2 = np.asarray(inputs["b2"], dtype=np.float32)
    W3 = np.asarray(inputs["W3"], dtype=np.float32)
    b3 = np.asarray(inputs["b3"], dtype=np.float32)

    in_maps = [
        _prep_member(x[e], W1[e], b1[e], W2[e], b2[e], W3[e], b3[e])
        for e in range(E)
    ]

    nc = _get_nc()
    res = run_bass_kernel_spmd(nc, in_maps, core_ids=list(range(E)))

    out = np.empty((E, B, OUT), dtype=np.float32)
    for e in range(E):
        dev = res.results[e]["out"]          # [128, HB] bf16
        out[e, :HB] = dev[0:OUT, :].T
        out[e, HB:] = dev[OUTP:OUTP + OUT, :].T
    return out

